# revision 45
# baseline (speedup 1.0000x reference)
"""Trainium2 Bass kernel for CustomGATConv (dense masked attention GNN layer).

  H = X @ W + b                       [8192, 64]
  S = H @ H.T ; S = where(A>0, S, -1e9)
  out = relu(softmax(S, -1) @ H)      [8192, 64]

Sharding: rows of the score matrix across 8 cores (1024 rows each).
Each core redundantly computes H (tiny) and processes its row block.

v7 design (from v3/v5/v6 traces):
  A HWDGE queue tops out near ~100 GB/s regardless of descriptor size,
  and non-16-bit masks wreck the e-multiply (u8 drops DVE to 1x and
  Pool's software mult costs 2.5us/tile -> v6's loop ran at mask-mult
  pace).  Bit-packing is out (STT chains only basic arith ops; one
  monotone scalar op can't extract two bits).  So masks stay bf16 and
  the DMA plan spreads them over the queues whose issue cost is free:
  v7 measured the gpsimd software-DGE at 127 GB/s with issue running on
  the idle gpsimd engine (~12.5us per 128-descriptor DMA), while
  scalar-ring issues burn ~4us EACH on the ACT engine (62us of exp time
  lost).  So:
  - masks [8, 128, 8192] bf16 (2MB groups, 16KB lines, 8 j-tiles each):
    groups 0,1,2,3,5,7 on the software-DGE ring (~160 GB/s sustained),
    groups 4,6 on the sync ring after the xt chunks.  The scalar ring
    issues only wb/adiag.  Zero DMA issues during the loop.
  - e-multiply: one DVE 2x tensor_mul per j (0.65us).
  - the diag clamp shrinks to the 128 columns that can actually hold a
    diagonal element per j<8 tile ([128,128] instead of [128,1024]).
  - H chunks (incl. the partition-64..127 duplicate via a second matmul
    at tile_position col 64 -- no SBUF-to-SBUF dup DMAs) interleave with
    the attention loop; PSUM-read copies split ACT (early chunks) / DVE.
  - out matmuls software-pipelined one pair behind the score matmuls so
    the PE never waits on exp/mask of the current pair.
  - ACT does exp only: 64 x [128,1024] ~ 73us, the steady-state pacer.
"""

import sys
import numpy as np

for _p in ("/opt/trn_rl_repo",):
    if _p not in sys.path:
        sys.path.insert(0, _p)

import concourse.bass as bass
import concourse.tile as tile
from concourse import bacc, mybir
from concourse.bass_utils import run_bass_kernel_spmd
from concourse.masks import make_identity

N = 8192          # nodes
D = 200           # in dim
F = 64            # out dim
NCORES = 8
M = N // NCORES   # 1024 rows per core
P = 128           # partitions
C_SHIFT = 64.0    # global softmax shift for off-diagonal scores

f32 = mybir.dt.float32
bf16 = mybir.dt.bfloat16
f16 = mybir.dt.float16
i16 = mybir.dt.int16
i32 = mybir.dt.int32
AF = mybir.ActivationFunctionType
ALU = mybir.AluOpType


def build_kernel(nc, outT, xt, wb, am, adiag):
    from contextlib import ExitStack

    with ExitStack() as ctx:
        tc = nc._tc
        const = ctx.enter_context(tc.tile_pool(name="const", bufs=1))
        fix = ctx.enter_context(tc.tile_pool(name="fix", bufs=1))
        atp = ctx.enter_context(tc.tile_pool(name="at", bufs=2))
        ep = ctx.enter_context(tc.tile_pool(name="ep", bufs=4))
        psP = ctx.enter_context(tc.tile_pool(name="ps", bufs=3, space="PSUM"))

        hbc = [const.tile([P, 512], f16, tag=f"hb{k}", name=f"hb{k}")
               for k in range(16)]          # H.T fp16, rows 64..127 duplicate
        hsbc = [const.tile([P, 4 * (F + 1)], bf16, tag=f"hs{k}", name=f"hs{k}")
                for k in range(16)]         # per j-tile row-major [H_j | 1]
        hto = const.tile([F, M], f32)       # own-rows H, fp32 (diag path)
        cbias = const.tile([P, 1], f32)     # -C bias for the exp
        ident = const.tile([F, F], f16)     # PE-transpose identity
        xt1c = [const.tile([P, 2048], f16, tag=f"x1{c}", name=f"x1{c}")
                for c in range(4)]
        xt2c = [const.tile([D + 1 - P, 2048], f16, tag=f"x2{c}", name=f"x2{c}")
                for c in range(4)]
        wt1 = const.tile([P, F], f16)
        wt2 = const.tile([D + 1 - P, F], f16)

        # ---- ring issue order.  gpsimd SWDGE: mask groups 0,1,2,3,5,7
        # ---- (2MB, 16KB lines, 8 j-tiles each) from t=0.  sync: the four
        # ---- xt column-chunk pairs, then groups 4, 6 and outT.  scalar:
        # ---- only wb/adiag (its issues run ON the ACT engine).
        at_tiles = {}

        for g in range(4):
            a = atp.tile([P, 16 * M], bf16, tag="at", name=f"at{g % 2}")
            nc.gpsimd.dma_start(a[:], am[g])
            at_tiles[g] = a
        nc.scalar.dma_start(wt1[:], wb[0:P, :])
        nc.scalar.dma_start(wt2[:], wb[P : D + 1, :])
        adi = fix.tile([1, M], i32)
        nc.scalar.dma_start(adi[:], adiag[:])
        for c in range(4):
            nc.sync.dma_start(xt1c[c][:], xt[0:P, bass.ts(c, 2048)])
            nc.sync.dma_start(xt2c[c][:], xt[P : D + 1, bass.ts(c, 2048)])

        nc.vector.memset(cbias[:], -C_SHIFT)
        make_identity(nc, ident[:])
        for k in range(16):
            h3 = hsbc[k][:].rearrange("p (a b) -> p a b", b=F + 1)
            nc.vector.memset(h3[:, :, F : F + 1], 1.0)

        # ---- phase 1 chunk emitters (chunk c = H.T cols 512c..512c+511) ----
        def emit_chunk_mm(c):
            ps = psP.tile([P, 512], f32, tag="ps", name=f"h{c}")
            c4, s = c // 4, bass.ts(c % 4, 512)
            # rows 0..63 and the 64..127 duplicate, straight from the PE
            nc.tensor.matmul(ps[0:F, :], wt1[:], xt1c[c4][:, s],
                             start=True, stop=False)
            nc.tensor.matmul(ps[0:F, :], wt2[:], xt2c[c4][:, s],
                             start=False, stop=True)
            nc.tensor.matmul(ps[F : 2 * F, :], wt1[:], xt1c[c4][:, s],
                             start=True, stop=False)
            nc.tensor.matmul(ps[F : 2 * F, :], wt2[:], xt2c[c4][:, s],
                             start=False, stop=True)
            # early chunks ride the pre-loop-idle ACT engine, later ones DVE
            if c < 4:
                nc.scalar.copy(hbc[c][:], ps[:])
            else:
                nc.vector.tensor_copy(hbc[c][:], ps[:])
            if c < 2:
                nc.scalar.copy(hto[:, bass.ts(c, 512)], ps[0:F, :])

        def emit_chunk_tr(c):
            trp = psP.tile([P, 4 * F], f16, tag="ps", name=f"t{c}")
            for q in range(4):
                nc.tensor.transpose(trp[:, bass.ts(q, F)],
                                    hbc[c][0:F, bass.ts(q, P)], ident[:])
            h3 = hsbc[c][:].rearrange("p (a b) -> p a b", b=F + 1)
            p3 = trp[:].rearrange("p (a b) -> p a b", b=F)
            if c < 4:
                nc.scalar.copy(h3[:, :, 0:F], p3[:])
            else:
                nc.vector.tensor_copy(h3[:, :, 0:F], p3[:])

        for c in range(4):
            emit_chunk_mm(c)
        for c in range(4):
            emit_chunk_tr(c)

        # ---- diag-score prep: d_r = |h_r|^2, merge scales ----
        htsq = fix.tile([F, M], f32)
        nc.vector.tensor_mul(htsq[:], hto[:], hto[:])
        ones64 = fix.tile([F, 1], f32)
        nc.vector.memset(ones64[:], 1.0)
        dsq = fix.tile([1, M], f32)
        for hi in range(2):
            psd = psP.tile([P, 512], f32, tag="ps", name=f"dsq{hi}")
            nc.tensor.matmul(psd[0:1, :], ones64[:], htsq[:, bass.ts(hi, 512)],
                             start=True, stop=True)
            nc.vector.tensor_copy(dsq[:, bass.ts(hi, 512)], psd[0:1, :])
        ad = fix.tile([1, M], f32)
        nc.vector.tensor_copy(ad[:], adi[:])
        # t1 = a*(d - C + 100) - 100  (== d-C where diag present, else -100)
        t1 = fix.tile([1, M], f32)
        nc.vector.scalar_tensor_tensor(t1[:], dsq[:], 100.0 - C_SHIFT, ad[:],
                                       ALU.add, ALU.mult)
        nc.vector.tensor_scalar_add(t1[:], t1[:], -100.0)
        mmx = fix.tile([1, M], f32)
        nc.vector.tensor_scalar_max(mmx[:], t1[:], 0.0)
        scm = fix.tile([1, M], f32)   # e^{-m}: scale for the off-diag partials
        nc.scalar.activation(scm[:], mmx[:], AF.Exp, scale=-1.0)
        scd = fix.tile([1, M], f32)   # e^{t1-m}: scale for the diag term
        nc.vector.tensor_sub(scd[:], t1[:], mmx[:])
        nc.scalar.activation(scd[:], scd[:], AF.Exp)

        # ---- phase 2: attention loop; out matmuls pipelined one pair back --
        ps_out = ctx.enter_context(tc.tile_pool(name="po", bufs=1,
                                                space="PSUM"))
        po = ps_out.tile([F + 1, M], f32, tag="po", name="po")
        HALVES = (slice(0, 512), slice(512, M))
        pending = None

        def flush_pending(sp):
            j0, lh0, e0, lh1, e1 = pending
            for j, lh, e in ((j0, lh0, e0), (j0 + 1, lh1, e1)):
                st = j == 0
                for half in HALVES:
                    nc.tensor.matmul(po[:, half], lh, e[:, half],
                                     start=st, stop=sp and j == j0 + 1,
                                     skip_group_check=True)

        for q in range(16):
            if q + 4 < 16:
                emit_chunk_mm(q + 4)
            at = at_tiles[q // 4]
            for hp in range(2):
                j0 = 4 * q + 2 * hp
                l0 = hbc[q][0:F, bass.ts(2 * hp, P)]
                l1 = hbc[q][F : 2 * F, bass.ts(2 * hp + 1, P)]
                ps0 = psP.tile([P, M], f32, tag="ps", name="ps0")
                ps1 = psP.tile([P, M], f32, tag="ps", name="ps1")
                for hi in range(2):
                    nc.tensor.matmul(ps0[:, bass.ts(hi, 512)], l0,
                                     hbc[hi][0:F, :], start=True, stop=True,
                                     tile_position=(0, 0))
                    nc.tensor.matmul(ps1[:, bass.ts(hi, 512)], l1,
                                     hbc[hi][F : 2 * F, :], start=True,
                                     stop=True, tile_position=(64, 0))
                if pending is not None:
                    flush_pending(False)
                pair = []
                for dk, psx in ((0, ps0), (1, ps1)):
                    j = j0 + dk
                    if q < 2:
                        # diag scores (|h_r|^2, up to ~190) would overflow
                        # bf16 after exp; off-diag max is ~99.6 so only the
                        # 128 columns that hold a diagonal element need the
                        # clamp (mask zeroes it; the tail re-adds exactly)
                        dcol = bass.ts(j, P)
                        nc.vector.tensor_scalar_min(psx[:, dcol],
                                                    psx[:, dcol], 150.0)
                    e = ep.tile([P, M], bf16, tag="e", name="e")
                    nc.scalar.activation(e[:], psx[:], AF.Exp, bias=cbias[:])
                    nc.vector.tensor_mul(e[:], e[:], at[:, bass.ts(j % 16, M)])
                    lh = hsbc[q][:, (j % 4) * (F + 1) : (j % 4 + 1) * (F + 1)]
                    pair += [lh, e]
                pending = (j0, *pair)
            if q + 4 < 16:
                emit_chunk_tr(q + 4)
        flush_pending(True)

        # ---- phase 3: merge + normalize ----
        posb = fix.tile([F, M], f32)
        nc.scalar.copy(posb[:], po[0:F, :])
        esum = fix.tile([1, M], f32)
        nc.vector.tensor_copy(esum[:], po[F : F + 1, :])
        den = fix.tile([1, M], f32)
        nc.vector.tensor_mul(den[:], esum[:], scm[:])
        nc.vector.tensor_add(den[:], den[:], scd[:])
        rden = fix.tile([1, M], f32)
        nc.vector.reciprocal_approx_fast(rden[:], den[:])
        alpha = fix.tile([1, M], f32)
        nc.vector.tensor_mul(alpha[:], scm[:], rden[:])
        beta = fix.tile([1, M], f32)
        nc.vector.tensor_mul(beta[:], scd[:], rden[:])

        # broadcast alpha/beta across 64 partitions via K=1 matmul with ones.
        # posb and htsq (dead by now) serve as the result scratch tiles.
        ones_row = fix.tile([1, F], f32)
        nc.vector.memset(ones_row[:], 1.0)
        for hi, half in enumerate(HALVES):
            ab = psP.tile([P, 512], f32, tag="ps", name=f"ab{hi}")
            nc.tensor.matmul(ab[0:F, :], ones_row[:], alpha[:, half],
                             start=True, stop=True)
            nc.vector.tensor_mul(posb[:, half], posb[:, half], ab[0:F, :])
            bb = psP.tile([P, 512], f32, tag="ps", name=f"bb{hi}")
            nc.tensor.matmul(bb[0:F, :], ones_row[:], beta[:, half],
                             start=True, stop=True)
            nc.vector.tensor_mul(htsq[:, half], hto[:, half], bb[0:F, :])
        nc.vector.tensor_add(posb[:], posb[:], htsq[:])
        nc.scalar.activation(posb[:], posb[:], AF.Relu)
        nc.sync.dma_start(outT[:], posb[:])


_NC_CACHE = {}


def get_compiled():
    if "nc" not in _NC_CACHE:
        nc = bacc.Bacc("TRN2", target_bir_lowering=False, debug=False,
                       enable_asserts=True, num_devices=NCORES)
        xt = nc.dram_tensor("xt", [D + 1, N], f16, kind="ExternalInput").ap()
        wb = nc.dram_tensor("wb", [D + 1, F], f16, kind="ExternalInput").ap()
        am = nc.dram_tensor("am", [4, P, 16 * M], bf16,
                            kind="ExternalInput").ap()
        adiag = nc.dram_tensor("adiag", [1, M], i32, kind="ExternalInput").ap()
        outT = nc.dram_tensor("outT", [F, M], f32, kind="ExternalOutput").ap()
        with tile.TileContext(nc) as tc:
            nc._tc = tc
            build_kernel(nc, outT, xt, wb, am, adiag)
        nc.compile()
        _NC_CACHE["nc"] = nc
    return _NC_CACHE["nc"]


def make_in_maps(X, A, W, b):
    import ml_dtypes
    X = np.asarray(X, dtype=np.float32)
    A = np.asarray(A)
    if A.dtype != np.int32:
        A = A.astype(np.int32)
    W = np.asarray(W, dtype=np.float32)
    b = np.asarray(b, dtype=np.float32).reshape(1, F)
    wb = np.ascontiguousarray(
        np.concatenate([W, b], axis=0).astype(np.float16))      # [201, 64]
    XT = np.concatenate([X.T, np.ones((1, N), np.float32)],
                        axis=0).astype(np.float16)              # [201, N]
    rng = np.arange(M)
    in_maps = []
    for c in range(NCORES):
        r0 = c * M
        xt_c = np.ascontiguousarray(np.roll(XT, -r0, axis=1))
        blk = np.roll(A[r0 : r0 + M], -r0, axis=1).copy()       # [M, N]
        adiag = blk[rng, rng].reshape(1, M).astype(np.int32)
        blk[rng, rng] = 0            # diag handled exactly by the fp32 tail
        # [4, 128, 16384] bf16 {0,1}: group g lane u holds j = 16g+u
        bits = ((blk.T.reshape(4, 16, P, M) > 0).astype(np.uint16) * 0x3F80)
        am = np.ascontiguousarray(bits.transpose(0, 2, 1, 3)
                                  .reshape(4, P, 16 * M)).view(
                                      ml_dtypes.bfloat16)
        in_maps.append({"xt": xt_c, "wb": wb, "am": am, "adiag": adiag})
    return in_maps


def kernel(X, A, W, b):
    nc = get_compiled()
    in_maps = make_in_maps(X, A, W, b)
    res = run_bass_kernel_spmd(nc, in_maps, list(range(NCORES)))
    outTs = [res.results[c]["outT"] for c in range(NCORES)]
    return np.ascontiguousarray(np.concatenate(outTs, axis=1).T)
